# revision 19
# baseline (speedup 1.0000x reference)
"""CrossAttentionBlock on 8 trn2 NeuronCores.

Sharding (per the hint): data parallel over batch B=2, tensor parallel over
heads (16 heads -> 4 groups of 4). Core c = b*4 + hg.

Phase A (8 cores): per (b, head-group) compute q/k/v projections for the
group's 256 dims, then masked softmax(QK^T/sqrt(d))V per head, normalized.
Everything is kept transposed ([dim x seq]) so the tensor engine contracts
along partitions at every step:
  - scores^T[kv, q] = kT[d, kv].T @ qT[d, q]           (K=64 per head)
  - exp evacuates scores PSUM->SBUF in one ACT pass per tile
  - mask folded multiplicatively into V (and into the denominator via a
    mask column appended to V), so exp needs no bias and no row max:
    scores are N(0,1)-scale so exp never overflows fp32.
  - attnV^T + denominator in one matmul per kv-tile via the [v | mask]
    65-column stationary operand.
Output: attnT (256 x 1024) per core, already normalized.

Phase B (8 cores): rows sharded (256 rows of B*SQ each): out = attn @ Wo.T
+ bo + residual, then LayerNorm. attnT from phase A is exactly the lhsT the
out-projection needs.

All matmuls run as float32r (full-rate fp32 on the PE; ~1e-4 rel err
measured on HW vs fp64-exact numpy).
"""

import numpy as np
from contextlib import ExitStack

import concourse.bacc as bacc
import concourse.tile as tile
import concourse.mybir as mybir
from concourse.bass_utils import run_bass_kernel_spmd

F32 = mybir.dt.float32
F32R = mybir.dt.float32r
AF = mybir.ActivationFunctionType
ALU = mybir.AluOpType
AX = mybir.AxisListType

B, SQ, SKV, E = 2, 1024, 4096, 1024
H, D = 16, 64
HG = 4                 # heads per core
HD = HG * D            # 256
P = 128
NE = E // P            # 8
NKV = SKV // P         # 32
LN_EPS = 1e-5
SCALE = 1.0 / np.sqrt(D)

_CACHE = {}


def _build_phase_a(ck_bufs=2, ex_bufs=4, sc_bufs=4, at_bufs=2, pj_bufs=2, n_streams=4, sc2_bufs=3, sm_bufs=2, n_proj=None):
    nc = bacc.Bacc("TRN2", target_bir_lowering=False, debug=False, num_devices=8)

    qT_d = nc.dram_tensor("qT", [E, SQ], F32R, kind="ExternalInput")
    kvT_d = nc.dram_tensor("kvT", [E, SKV], F32R, kind="ExternalInput")
    wqT_d = nc.dram_tensor("wqT", [E, HD], F32R, kind="ExternalInput")
    wkT_d = nc.dram_tensor("wkT", [E, HD], F32R, kind="ExternalInput")
    wvT_d = nc.dram_tensor("wvT", [E, HD], F32R, kind="ExternalInput")
    bq_d = nc.dram_tensor("bq", [1, HD], F32R, kind="ExternalInput")
    bk_d = nc.dram_tensor("bk", [1, HD], F32R, kind="ExternalInput")
    bv_d = nc.dram_tensor("bv", [1, HD], F32R, kind="ExternalInput")
    mask_d = nc.dram_tensor("mask01", [P, NKV], F32, kind="ExternalInput")
    attnT_d = nc.dram_tensor("attnT", [HD, SQ], F32, kind="ExternalOutput")

    with tile.TileContext(nc) as tc, ExitStack() as ctx:
        const = ctx.enter_context(tc.tile_pool(name="const", bufs=1))

        wq_sb = const.tile([P, NE, HD], F32R)
        nc.sync.dma_start(wq_sb[:], wqT_d.ap().rearrange("(j p) d -> p j d", p=P))
        bq_sb = const.tile([1, HD], F32R)
        nc.sync.dma_start(bq_sb[:], bq_d.ap())
        wk_sb = const.tile([P, NE, HD], F32R)
        nc.sync.dma_start(wk_sb[:], wkT_d.ap().rearrange("(j p) d -> p j d", p=P))
        wv_sb = const.tile([P, NE, HD], F32R)
        nc.sync.dma_start(wv_sb[:], wvT_d.ap().rearrange("(j p) d -> p j d", p=P))
        bk_sb = const.tile([1, HD], F32R)
        nc.sync.dma_start(bk_sb[:], bk_d.ap())
        bv_sb = const.tile([1, HD], F32R)
        nc.sync.dma_start(bv_sb[:], bv_d.ap())
        mask_sb = const.tile([P, NKV], F32)
        nc.sync.dma_start(mask_sb[:], mask_d.ap())
        ones32_sb = const.tile([1, 512], F32)
        nc.any.memset(ones32_sb[:], 1.0)
        ones_sb = const.tile([1, 512], F32R)
        nc.vector.tensor_copy(ones_sb[:], ones32_sb[:])

        # per-chunk product tiles: attention on kv-tile t depends only on
        # chunk t//4's tiles, so it can overlap later projection chunks
        # (one big tile would serialize attention behind the last chunk).
        qTs_sb = const.tile([P, 2, SQ], F32R)
        attnT_sb = const.tile([P, 2, SQ], F32)
        kT_c = [const.tile([P, 2, 512], F32R, name=f"kTc{c}") for c in range(8)]
        v_c = [const.tile([P, 4, HG * (D + 1)], F32R, name=f"vc{c}") for c in range(8)]

        # mask columns of v_aug: v[:, tt, 65*h + 64] = mask[:, 4c+tt]
        maskr_sb = const.tile([P, NKV], F32R)
        nc.vector.tensor_copy(maskr_sb[:], mask_sb[:])
        for c in range(8):
            for h in range(HG):
                nc.vector.tensor_copy(v_c[c][:, :, h * (D + 1) + D],
                                      maskr_sb[:, 4 * c:4 * c + 4])

        # ---------------- pools. Two regions:
        #  R1: projections with attention stream (0,0) interleaved.
        #      PSUM: accum 2 + scores512 x4 = 4 + proj 2 = 8 banks.
        #  R2: attention streams (0,1),(1,0),(1,1), 1024-wide scores tiles.
        #      PSUM: accum 2 + scores1024 x3 = 6 -> 8 banks.
        at_ps = ctx.enter_context(tc.tile_pool(name="atps", bufs=at_bufs, space="PSUM"))
        ex_pool = ctx.enter_context(tc.tile_pool(name="expool", bufs=ex_bufs))
        sm_pool = ctx.enter_context(tc.tile_pool(name="smpool", bufs=sm_bufs))

        def do_q_chunk(c, ck_pool, pj_ps):
            ch = ck_pool.tile([P, NE, 512], F32R, tag="ch", name=f"chq{c}")
            for j in range(NE):
                nc.sync.dma_start(ch[:, j, :], qT_d.ap()[j * P:(j + 1) * P, c * 512:(c + 1) * 512])
            for m in range(2):
                ps = pj_ps.tile([P, 512], F32, tag="pj", name=f"qps{c}_{m}")
                for j in range(NE):
                    nc.tensor.matmul(ps[:], wq_sb[:, j, m * P:(m + 1) * P], ch[:, j, :],
                                     start=(j == 0), stop=False)
                nc.tensor.matmul(ps[:], bq_sb[:, m * P:(m + 1) * P], ones_sb[:],
                                 start=False, stop=True)
                nc.vector.tensor_copy(qTs_sb[:, m, c * 512:(c + 1) * 512], ps[:])

        def do_kv_chunk(c, ck_pool, pj_ps):
            ch = ck_pool.tile([P, NE, 512], F32R, tag="ch", name=f"chkv{c}")
            for j in range(NE):
                nc.sync.dma_start(ch[:, j, :], kvT_d.ap()[j * P:(j + 1) * P, c * 512:(c + 1) * 512])
            for m in range(2):
                ps = pj_ps.tile([P, 512], F32, tag="pj", name=f"kps{c}_{m}")
                for j in range(NE):
                    nc.tensor.matmul(ps[:], wk_sb[:, j, m * P:(m + 1) * P], ch[:, j, :],
                                     start=(j == 0), stop=False)
                nc.tensor.matmul(ps[:], bk_sb[:, m * P:(m + 1) * P], ones_sb[:],
                                 start=False, stop=True)
                nc.vector.tensor_copy(kT_c[c][:, m, :], ps[:])
            for tt in range(4):
                t = 4 * c + tt
                ps = pj_ps.tile([P, HD], F32, tag="pj", name=f"vps{t}")
                for j in range(NE):
                    nc.tensor.matmul(ps[:], ch[:, j, tt * P:(tt + 1) * P], wv_sb[:, j, :],
                                     start=(j == 0), stop=False)
                nc.tensor.matmul(ps[:], ones_sb[:, 0:P], bv_sb[:], start=False, stop=True)
                nc.vector.tensor_scalar(
                    v_c[c][:, tt, :].rearrange("p (h u) -> p h u", u=D + 1)[:, :, 0:D],
                    ps[:].rearrange("p (h u) -> p h u", u=D),
                    mask_sb[:, t:t + 1], None, op0=ALU.mult)

        def att_scores_512(sc_pool, m, qh, t, hh):
            ps = sc_pool.tile([P, 512], F32, tag="sc", name=f"s{m}{qh}_{t}_{hh}")
            nc.tensor.matmul(
                ps[:],
                kT_c[t // 4][hh * D:(hh + 1) * D, m, (t % 4) * P:(t % 4 + 1) * P],
                qTs_sb[hh * D:(hh + 1) * D, m, qh * 512:(qh + 1) * 512],
                start=True, stop=True)
            ex = ex_pool.tile([P, 512], F32R, tag="ex", name=f"e{m}{qh}_{t}_{hh}")
            nc.scalar.activation(ex[:], ps[:], AF.Exp, scale=float(SCALE))
            return ex

        def att_scores_1024(sc_pool, m, qh, t):
            ps = sc_pool.tile([P, 1024], F32, tag="sc", name=f"s{m}{qh}_{t}")
            for hh in range(2):
                nc.tensor.matmul(
                    ps[:, hh * 512:(hh + 1) * 512],
                    kT_c[t // 4][hh * D:(hh + 1) * D, m, (t % 4) * P:(t % 4 + 1) * P],
                    qTs_sb[hh * D:(hh + 1) * D, m, qh * 512:(qh + 1) * 512],
                    start=True, stop=True)
            ex = ex_pool.tile([P, 1024], F32R, tag="ex", name=f"e{m}{qh}_{t}")
            nc.scalar.activation(ex[:], ps[:], AF.Exp, scale=float(SCALE))
            return ex

        def att_av(m, qh, t, ex, exoff, hh, pv):
            h = 2 * m + hh
            nc.tensor.matmul(
                pv[hh][:],
                v_c[t // 4][:, t % 4, h * (D + 1):(h + 1) * (D + 1)],
                ex[:, exoff:exoff + 512],
                start=(t == 0), stop=(t == NKV - 1))

        def att_norm(sc_pool, m, qh, pv):
            for hh in range(2):
                den = sm_pool.tile([1, 512], F32, tag="den", name=f"den{m}{qh}{hh}")
                nc.vector.tensor_copy(den[:], pv[hh][D:D + 1, :])
                rec = sm_pool.tile([1, 512], F32R, tag="rec", name=f"rec{m}{qh}{hh}")
                with nc.allow_low_precision(reason="recip feeds f32r matmul; PE rounds inputs anyway"):
                    nc.vector.reciprocal(rec[:], den[:])
                raw = sm_pool.tile([D, 512], F32, tag="raw", name=f"raw{m}{qh}{hh}")
                nc.vector.tensor_copy(raw[:], pv[hh][0:D, :])
                bc = sc_pool.tile([D, 512], F32, tag="sc", name=f"bc{m}{qh}{hh}")
                nc.tensor.matmul(bc[:], ones_sb[:, 0:D], rec[:], start=True, stop=True)
                nc.vector.tensor_tensor(
                    attnT_sb[hh * D:(hh + 1) * D, m, qh * 512:(qh + 1) * 512],
                    raw[:], bc[:], op=ALU.mult)

        def new_pv(m, qh):
            return [at_ps.tile([D + 1, 512], F32, tag="acc", name=f"pv{m}{qh}_{hh}")
                    for hh in range(2)]

        # ---- region 1: projections with streams (0,0),(0,1) interleaved.
        # 1-(tile,head) emission skew keeps the PE ahead of exp.
        with ExitStack() as r1:
            sc_r1 = r1.enter_context(tc.tile_pool(name="scr1", bufs=sc_bufs, space="PSUM"))
            ck_pool = r1.enter_context(tc.tile_pool(name="ck", bufs=ck_bufs))
            pj_ps = r1.enter_context(tc.tile_pool(name="pjps", bufs=pj_bufs, space="PSUM"))

            do_q_chunk(0, ck_pool, pj_ps)
            do_q_chunk(1, ck_pool, pj_ps)
            pv0 = new_pv(0, 0)
            pending = None
            for c in range(SKV // 512):
                do_kv_chunk(c, ck_pool, pj_ps)
                if n_streams == 0:
                    continue
                for t in range(4 * c, 4 * c + 4):
                    for hh in range(2):
                        ex = att_scores_512(sc_r1, 0, 0, t, hh)
                        if pending is not None:
                            att_av(0, *pending)
                        pending = (0, t, ex, 0, hh, pv0)
            if n_streams > 0:
                att_av(0, *pending)
                att_norm(sc_r1, 0, 0, pv0)

        # ---- region 2: remaining streams, attention only, 1024-wide scores
        if n_streams > 1:
            with ExitStack() as r2:
                sc_r2 = r2.enter_context(tc.tile_pool(name="scr2", bufs=sc2_bufs, space="PSUM"))
                for (m, qh) in ((0, 1), (1, 0), (1, 1))[:n_streams - 1]:
                    pv = new_pv(m, qh)
                    pending = None
                    for t in range(NKV):
                        ex = att_scores_1024(sc_r2, m, qh, t)
                        if pending is not None:
                            att_av(m, *pending)
                            att_av(m, *pending2)
                        pending = (qh, t, ex, 0, 0, pv)
                        pending2 = (qh, t, ex, 512, 1, pv)
                    att_av(m, *pending)
                    att_av(m, *pending2)
                    att_norm(sc_r2, m, qh, pv)

        nc.sync.dma_start(attnT_d.ap().rearrange("(m p) q -> p m q", p=P), attnT_sb[:])

    nc.compile()
    return nc


def _build_phase_b():
    R = 2 * P   # 256 rows per core
    nc = bacc.Bacc("TRN2", target_bir_lowering=False, debug=False, num_devices=8)

    aT_d = nc.dram_tensor("aT", [E, R], F32R, kind="ExternalInput")
    woT_d = nc.dram_tensor("woT", [E, E], F32R, kind="ExternalInput")
    qn_d = nc.dram_tensor("qn", [R, E], F32, kind="ExternalInput")
    bo_d = nc.dram_tensor("bo", [1, E], F32R, kind="ExternalInput")
    gam_d = nc.dram_tensor("gam", [1, E], F32R, kind="ExternalInput")
    bet_d = nc.dram_tensor("bet", [1, E], F32R, kind="ExternalInput")
    y_d = nc.dram_tensor("y", [R, E], F32, kind="ExternalOutput")

    with tile.TileContext(nc) as tc, ExitStack() as ctx:
        const = ctx.enter_context(tc.tile_pool(name="const", bufs=1))
        aT_sb = const.tile([P, NE, R], F32R)
        for k in range(NE):
            nc.sync.dma_start(aT_sb[:, k, :], aT_d.ap()[k * P:(k + 1) * P, :])
        qn_sb = const.tile([P, 2, E], F32)
        for mt in range(2):
            nc.sync.dma_start(qn_sb[:, mt, :], qn_d.ap().rearrange("(m p) e -> p m e", p=P)[:, mt, :])
        bo_sb = const.tile([1, E], F32R)
        nc.sync.dma_start(bo_sb[:], bo_d.ap())
        gam_sb = const.tile([1, E], F32R)
        nc.sync.dma_start(gam_sb[:], gam_d.ap())
        bet_sb = const.tile([1, E], F32R)
        nc.sync.dma_start(bet_sb[:], bet_d.ap())
        ones32_sb = const.tile([1, P], F32)
        nc.any.memset(ones32_sb[:], 1.0)
        ones_sb = const.tile([1, P], F32R)
        nc.vector.tensor_copy(ones_sb[:], ones32_sb[:])

        gam_bc = const.tile([P, E], F32)
        bet_bc = const.tile([P, E], F32)

        wo_pool = ctx.enter_context(tc.tile_pool(name="wo", bufs=4))
        ps_pool = ctx.enter_context(tc.tile_pool(name="ps", bufs=2, space="PSUM"))
        gb_ps = ctx.enter_context(tc.tile_pool(name="gbps", bufs=2, space="PSUM"))
        sbp = ctx.enter_context(tc.tile_pool(name="sbp", bufs=2))

        ps_tiles = {}
        for k in range(NE):
            wo = wo_pool.tile([P, E], F32R, tag="wo", name=f"wo{k}")
            nc.sync.dma_start(wo[:], woT_d.ap()[k * P:(k + 1) * P, :])
            for mt in range(2):
                if k == 0:
                    ps_tiles[mt] = ps_pool.tile([P, E], F32, tag="o", name=f"o{mt}")
                for nh in range(2):
                    nc.tensor.matmul(ps_tiles[mt][:, nh * 512:(nh + 1) * 512],
                                     aT_sb[:, k, mt * P:(mt + 1) * P],
                                     wo[:, nh * 512:(nh + 1) * 512],
                                     start=(k == 0), stop=False)
        for mt in range(2):
            for nh in range(2):
                nc.tensor.matmul(ps_tiles[mt][:, nh * 512:(nh + 1) * 512], ones_sb[:],
                                 bo_sb[:, nh * 512:(nh + 1) * 512],
                                 start=False, stop=True)

        # broadcast gamma/beta rows to all 128 partitions via K=1 matmuls
        for half in range(2):
            cs = slice(half * 512, (half + 1) * 512)
            psg = gb_ps.tile([P, 512], F32, tag="gb", name=f"gbg{half}")
            nc.tensor.matmul(psg[:], ones_sb[:], gam_sb[:, cs], start=True, stop=True)
            nc.scalar.copy(gam_bc[:, cs], psg[:])
            psb = gb_ps.tile([P, 512], F32, tag="gb", name=f"gbb{half}")
            nc.tensor.matmul(psb[:], ones_sb[:], bet_sb[:, cs], start=True, stop=True)
            nc.scalar.copy(bet_bc[:, cs], psb[:])

        # residual + LayerNorm, var = E[x^2] - mean^2 so the two reductions
        # run on different engines (DVE reduce, ACT Square+accum) in parallel
        for mt in range(2):
            x = sbp.tile([P, E], F32, tag="x", name=f"x{mt}")
            nc.vector.tensor_tensor(x[:], ps_tiles[mt][:], qn_sb[:, mt, :], op=ALU.add)
            s1 = sbp.tile([P, 1], F32, tag="s1", name=f"s1{mt}")
            nc.vector.reduce_sum(s1[:], x[:], axis=AX.X)
            sq = sbp.tile([P, E], F32, tag="sq", name=f"sq{mt}")
            ssq = sbp.tile([P, 1], F32, tag="ssq", name=f"ssq{mt}")
            nc.scalar.activation(sq[:], x[:], AF.Square, accum_out=ssq[:])
            nm = sbp.tile([P, 1], F32, tag="nm", name=f"nm{mt}")
            nc.vector.tensor_scalar(nm[:], s1[:], -1.0 / E, None, op0=ALU.mult)
            m2 = sbp.tile([P, 1], F32, tag="m2", name=f"m2{mt}")
            nc.vector.tensor_tensor(m2[:], nm[:], nm[:], op=ALU.mult)
            var = sbp.tile([P, 1], F32, tag="var", name=f"var{mt}")
            nc.vector.tensor_scalar(var[:], ssq[:], 1.0 / E, LN_EPS, op0=ALU.mult, op1=ALU.add)
            nc.vector.tensor_tensor(var[:], var[:], m2[:], op=ALU.subtract)
            rv = sbp.tile([P, 1], F32, tag="rv", name=f"rv{mt}")
            nc.vector.reciprocal(rv[:], var[:])
            rstd = sbp.tile([P, 1], F32, tag="rstd", name=f"rstd{mt}")
            nc.scalar.activation(rstd[:], rv[:], AF.Sqrt)
            yn = sbp.tile([P, E], F32, tag="yn", name=f"yn{mt}")
            nc.vector.tensor_scalar(yn[:], x[:], nm[:], rstd[:], op0=ALU.add, op1=ALU.mult)
            yg = sbp.tile([P, E], F32, tag="yg", name=f"yg{mt}")
            nc.vector.tensor_tensor(yg[:], yn[:], gam_bc[:], op=ALU.mult)
            yb = sbp.tile([P, E], F32, tag="yb", name=f"yb{mt}")
            nc.vector.tensor_tensor(yb[:], yg[:], bet_bc[:], op=ALU.add)
            nc.sync.dma_start(y_d.ap().rearrange("(m p) e -> p m e", p=P)[:, mt, :], yb[:])

    nc.compile()
    return nc


def _get(name):
    if name not in _CACHE:
        _CACHE[name] = _build_phase_a() if name == "a" else _build_phase_b()
    return _CACHE[name]


def kernel(query, key_value, key_value_mask, Wq, bq, Wk, bk, Wv, bv, Wo, bo,
           ln_gamma, ln_beta):
    f = lambda a: np.ascontiguousarray(np.asarray(a, dtype=np.float32))
    query, key_value = f(query), f(key_value)
    Wq, Wk, Wv, Wo = f(Wq), f(Wk), f(Wv), f(Wo)
    bq, bk, bv, bo = f(bq), f(bk), f(bv), f(bo)
    ln_gamma, ln_beta = f(ln_gamma), f(ln_beta)
    mask01 = (np.asarray(key_value_mask) != 0).astype(np.float32)

    nc_a = _get("a")
    in_maps_a = []
    for c in range(8):
        b, hg = c // 4, c % 4
        sl = slice(hg * HD, (hg + 1) * HD)
        in_maps_a.append({
            "qT": f(query[b].T),
            "kvT": f(key_value[b].T),
            "wqT": f(Wq[sl].T),
            "wkT": f(Wk[sl].T),
            "wvT": f(Wv[sl].T),
            "bq": bq[sl].reshape(1, HD),
            "bk": bk[sl].reshape(1, HD),
            "bv": bv[sl].reshape(1, HD),
            "mask01": f(mask01[b].reshape(NKV, P).T),
        })
    res_a = run_bass_kernel_spmd(nc_a, in_maps_a, core_ids=list(range(8)))
    attnT = [np.concatenate([res_a.results[b * 4 + hg]["attnT"] for hg in range(4)], axis=0)
             for b in range(B)]   # per batch: (1024 dims, 1024 q)

    nc_b = _get("b")
    woT = f(Wo.T)
    bo_r = bo.reshape(1, E)
    gam_r = ln_gamma.reshape(1, E)
    bet_r = ln_beta.reshape(1, E)
    in_maps_b = []
    for c in range(8):
        b, j = c // 4, c % 4
        rs = slice(j * 256, (j + 1) * 256)
        in_maps_b.append({
            "aT": f(attnT[b][:, rs]),
            "woT": woT,
            "qn": f(query[b, rs, :]),
            "bo": bo_r,
            "gam": gam_r,
            "bet": bet_r,
        })
    res_b = run_bass_kernel_spmd(nc_b, in_maps_b, core_ids=list(range(8)))
    out = np.empty((B, SQ, E), np.float32)
    for c in range(8):
        b, j = c // 4, c % 4
        out[b, j * 256:(j + 1) * 256, :] = res_b.results[c]["y"]
    return out
